# revision 21
# baseline (speedup 1.0000x reference)
"""Sparse ConvTranspose3d (gather + GEMM + scatter-add) on 8 TRN2 NeuronCores.

Device strategy: the per-token dma_scatter_add path is hard-capped by tiny
256B RMW packets to HBM (~280ns/packet: measured 4.7ms SWDGE-DMA busy in the
scatter baseline) and by Q7 descriptor generation (2.1ms), so the device
instead computes, for its shard of active voxels, all 27 per-offset GEMM
contributions and streams them to DRAM as large contiguous bf16 writes at
full DMA bandwidth. The index-directed scatter-add is part of the host-side
unshard: for each kernel offset k the (deduped) output indices are unique
(translation injectivity), so the merge is 27 exact vectorized
fancy-index adds.

Sharding: active voxels split evenly across 8 cores; weight replicated.

Per-core device pipeline (~86-95us, vs 7.16ms for the dma_scatter_add
baseline): chunk pairs of 128 points ride the PE's two 64-row tiles
(contraction=64), eight 1-bank PSUM tiles deep-pipeline the 8x432-col
matmuls per pair, PSUM evacuation alternates DVE/ACT, and each pair
flushes one 884KB contiguous bf16 DMA; DMA runs at ~350GB/s, the HBM
line rate, and is the critical path.
"""
import numpy as np
import ml_dtypes

import concourse.bass as bass
import concourse.bacc as bacc
import concourse.tile as tile
import concourse.mybir as mybir
from concourse.bass_utils import run_bass_kernel_spmd

N_CORES = 8
KV = 27
CIN = 64
COUT = 64
WCOLS = KV * COUT                # 1728

_prog_cache = {}
_last = {}                       # test-only: program + in_maps of last call


def _build_program(NCH, style="v7"):
    """SPMD program: [2*CIN, NPTS/2] paired feats -> [NPTS, KV*COUT].

    Chunk pairs share SBUF columns: even chunk's features live on
    partitions 0-63, odd chunk's on 64-127, so the two matmuls run
    concurrently on the PE's two 64-row tiles (T0 / T8).

    style "v6": eight 1-bank PSUM tiles/pair, eight 432-col copies.
    style "v7": four 2-bank PSUM tiles/pair, four 1024/704-col copies
    (512-col matmuls stay bank-aligned).
    style "v8": v6 PSUM layout; odd NCH allowed (trailing singleton chunk
    on the T0 half), and the first/last pairs flush per-chunk DMAs to
    tighten pipeline ramp and drain.
    """
    if style != "v8":
        assert NCH % 2 == 0
    NPTS = NCH * 128
    NPAIR = NCH // 2
    FCOLS = -(-NCH // 2) * 128      # ft columns (pair blocks, rounded up)
    NSLICE = 4
    SL = WCOLS // NSLICE            # 432 cols -> one PSUM bank each
    nc = bacc.Bacc("TRN2", target_bir_lowering=False, debug=False,
                   enable_asserts=False, num_devices=N_CORES)
    ft = nc.dram_tensor("ft", [2 * CIN, FCOLS], mybir.dt.bfloat16,
                        kind="ExternalInput")
    wt = nc.dram_tensor("wt", [2 * CIN, WCOLS], mybir.dt.bfloat16,
                        kind="ExternalInput")
    work = nc.dram_tensor("work", [NPTS, WCOLS], mybir.dt.bfloat16,
                          kind="ExternalOutput")

    with tile.TileContext(nc) as tc:
        with (
            tc.tile_pool(name="const", bufs=1) as cpool,
            tc.tile_pool(name="cbuf", bufs=4) as cbpool,
            tc.tile_pool(name="psum", bufs=8 if style == "v6" else 4,
                         space="PSUM") as ppool,
        ):
            ft_t = cpool.tile([2 * CIN, FCOLS], mybir.dt.bfloat16)
            wt_t = cpool.tile([2 * CIN, WCOLS], mybir.dt.bfloat16)
            nc.sync.dma_start(out=wt_t[:], in_=wt[:])
            nseg = 6
            seg = -(-FCOLS // (128 * nseg)) * 128
            for s in range(nseg):
                s0, s1 = s * seg, min((s + 1) * seg, FCOLS)
                if s0 < s1:
                    nc.sync.dma_start(out=ft_t[:, s0:s1], in_=ft[:, s0:s1])

            for pr in range(NPAIR):
                cols = slice(pr * 128, (pr + 1) * 128)
                c_t = cbpool.tile([128, 2, WCOLS], mybir.dt.bfloat16)
                if style in ("v6", "v8"):
                    for j in range(4):
                        n = slice(j * SL, (j + 1) * SL)
                        for h in range(2):
                            hp = slice(h * CIN, (h + 1) * CIN)
                            ps = ppool.tile([128, SL], mybir.dt.float32,
                                            space="PSUM", tag="ps")
                            nc.tensor.matmul(
                                out=ps[:],
                                lhsT=ft_t[hp, cols],
                                rhs=wt_t[hp, n],
                                start=True, stop=True)
                            eng = (nc.vector.tensor_copy if (j + h) % 2 == 0
                                   else nc.scalar.copy)
                            eng(out=c_t[:, h, n], in_=ps[:])
                else:
                    for h in range(2):
                        hp = slice(h * CIN, (h + 1) * CIN)
                        for j in range(2):
                            w0 = j * 1024           # 1024 then 704 cols
                            w1 = min(w0 + 1024, WCOLS)
                            ps = ppool.tile([128, 1024], mybir.dt.float32,
                                            space="PSUM", tag="ps")
                            for m0 in range(w0, w1, 512):
                                m1 = min(m0 + 512, w1)
                                nc.tensor.matmul(
                                    out=ps[:, m0 - w0:m1 - w0],
                                    lhsT=ft_t[hp, cols],
                                    rhs=wt_t[hp, m0:m1],
                                    start=True, stop=True)
                            eng = (nc.vector.tensor_copy if (j + h) % 2 == 0
                                   else nc.scalar.copy)
                            eng(out=c_t[:, h, w0:w1], in_=ps[:, :w1 - w0])
                dst = work[pr * 256:(pr + 1) * 256, :].rearrange(
                    "(h q) c -> q h c", q=128)
                ends = (0, NPAIR - 1) if style == "v8" else (NPAIR,)
                if pr in ends:
                    # per-chunk DMAs at the pipeline ends: the first output
                    # flushes sooner and the drain waits on one chunk only
                    for h in range(2):
                        ch = 2 * pr + h
                        nc.sync.dma_start(
                            out=work[ch * 128:(ch + 1) * 128, :],
                            in_=c_t[:, h, :])
                elif pr < NPAIR - 1 or style == "v8":
                    nc.sync.dma_start(out=dst, in_=c_t[:])
                else:
                    # split the tail DMA so it starts before the last copies
                    nc.sync.dma_start(out=dst[:, :, :WCOLS // 2],
                                      in_=c_t[:, :, :WCOLS // 2])
                    nc.sync.dma_start(out=dst[:, :, WCOLS // 2:],
                                      in_=c_t[:, :, WCOLS // 2:])

            if style == "v8" and NCH % 2:
                # trailing singleton chunk rides the T0 half only
                ch = NCH - 1
                cols = slice(NPAIR * 128, (NPAIR + 1) * 128)
                c_t = cbpool.tile([128, 2, WCOLS], mybir.dt.bfloat16)
                for j in range(4):
                    n = slice(j * SL, (j + 1) * SL)
                    ps = ppool.tile([128, SL], mybir.dt.float32,
                                    space="PSUM", tag="ps")
                    nc.tensor.matmul(
                        out=ps[:],
                        lhsT=ft_t[:CIN, cols],
                        rhs=wt_t[:CIN, n],
                        start=True, stop=True)
                    eng = (nc.vector.tensor_copy if j % 2 == 0
                           else nc.scalar.copy)
                    eng(out=c_t[:, 0, n], in_=ps[:])
                    nc.sync.dma_start(
                        out=work[ch * 128:(ch + 1) * 128, n],
                        in_=c_t[:, 0, n])
    nc.compile()
    return nc


def kernel(feats, weight, bias, out_index, n_out):
    feats = np.asarray(feats, np.float32)
    weight = np.asarray(weight, np.float32)
    bias = np.asarray(bias, np.float32)
    oi = np.asarray(out_index, np.int32)
    n_out = int(n_out)

    # ---- merge duplicate-coordinate points (makes oi[k] unique per k) ----
    order = np.argsort(oi[0], kind="stable")
    b0 = oi[0][order]
    dup = np.zeros(len(order), bool)
    dup[1:] = b0[1:] == b0[:-1]
    heads = np.where(~dup, np.arange(len(order)), 0)
    np.maximum.accumulate(heads, out=heads)
    f_s = feats[order].copy()
    if dup.any():
        np.add.at(f_s, heads[dup], f_s[np.flatnonzero(dup)])
    keep = ~dup
    f_s = f_s[keep]
    oi_s = oi[:, order[keep]]                   # [27, M], unique per k
    M = oi_s.shape[1]

    # ---- shard points evenly across cores ----
    cnt = [(M + N_CORES - 1 - c) // N_CORES for c in range(N_CORES)]
    starts = np.cumsum([0] + cnt)
    NCH = -(-max(cnt) // 128)
    FPTS = -(-NCH // 2) * 256       # points padded to full chunk pairs

    if NCH not in _prog_cache:
        _prog_cache[NCH] = _build_program(NCH, "v8")
    nc = _prog_cache[NCH]

    # rhs[c, k*64+o] = weight[k, o, c]; duplicated on partitions 64-127
    wt_half = np.ascontiguousarray(
        weight.transpose(2, 0, 1).reshape(CIN, WCOLS)).astype(
            ml_dtypes.bfloat16)
    wt_np = np.concatenate([wt_half, wt_half])
    in_maps = []
    for c in range(N_CORES):
        fpad = np.zeros((FPTS, CIN), ml_dtypes.bfloat16)
        fpad[:cnt[c]] = f_s[starts[c]:starts[c + 1]].astype(ml_dtypes.bfloat16)
        # [npair, 2, 128, CIN] -> [2*CIN, npair * 128]
        ft_np = np.ascontiguousarray(
            fpad.reshape(FPTS // 256, 2, 128, CIN).transpose(1, 3, 0, 2)
            .reshape(2 * CIN, FPTS // 2))
        in_maps.append({"ft": ft_np, "wt": wt_np})

    res = run_bass_kernel_spmd(nc, in_maps, list(range(N_CORES)))
    _last["nc"] = nc
    _last["in_maps"] = in_maps

    # ---- host unshard: 27 exact per-offset merges + bias ----
    contrib = np.concatenate(
        [np.asarray(res.results[c]["work"])[:cnt[c]].reshape(cnt[c], KV, COUT)
         for c in range(N_CORES)])                 # [M, 27, 64] bf16
    out = np.empty((n_out, COUT), np.float32)
    out[:] = bias
    for k in range(KV):
        out[oi_s[k]] += contrib[:, k].astype(np.float32)
    return out


# revision 23
# speedup vs baseline: 1.2854x; 1.2854x over previous
"""Sparse ConvTranspose3d (gather + GEMM + scatter-add) on 8 TRN2 NeuronCores.

Device strategy: the per-token dma_scatter_add path is hard-capped by tiny
256B RMW packets to HBM (~280ns/packet: measured 4.7ms SWDGE-DMA busy in the
scatter baseline) and by Q7 descriptor generation (2.1ms), so the device
instead computes, for its shard of active voxels, all 27 per-offset GEMM
contributions and streams them to DRAM as large contiguous bf16 writes at
full DMA bandwidth. The index-directed scatter-add is part of the host-side
unshard: for each kernel offset k the (deduped) output indices are unique
(translation injectivity), so the merge is 27 exact vectorized
fancy-index adds.

Sharding: active voxels split evenly across 8 cores; weight replicated.

Per-core device pipeline (~86-95us, vs 7.16ms for the dma_scatter_add
baseline): chunk pairs of 128 points ride the PE's two 64-row tiles
(contraction=64), eight 1-bank PSUM tiles deep-pipeline the 8x432-col
matmuls per pair, PSUM evacuation alternates DVE/ACT, and each pair
flushes one 884KB contiguous bf16 DMA; DMA runs at ~350GB/s, the HBM
line rate, and is the critical path.
"""
import numpy as np
import ml_dtypes

import concourse.bass as bass
import concourse.bacc as bacc
import concourse.tile as tile
import concourse.mybir as mybir
from concourse.bass_utils import run_bass_kernel_spmd

N_CORES = 8
KV = 27
CIN = 64
COUT = 64
WCOLS = KV * COUT                # 1728

_prog_cache = {}
_last = {}                       # test-only: program + in_maps of last call


def _build_program(NCH, style="v7"):
    """SPMD program: [2*CIN, NPTS/2] paired feats -> [NPTS, KV*COUT].

    Chunk pairs share SBUF columns: even chunk's features live on
    partitions 0-63, odd chunk's on 64-127, so the two matmuls run
    concurrently on the PE's two 64-row tiles (T0 / T8).

    style "v6": eight 1-bank PSUM tiles/pair, eight 432-col copies.
    style "v7": four 2-bank PSUM tiles/pair, four 1024/704-col copies
    (512-col matmuls stay bank-aligned).
    style "v8": v6 PSUM layout; odd NCH allowed (trailing singleton chunk
    on the T0 half), and the first/last pairs flush per-chunk DMAs to
    tighten pipeline ramp and drain.
    """
    if style != "v8":
        assert NCH % 2 == 0
    NPTS = NCH * 128
    NPAIR = NCH // 2
    FCOLS = -(-NCH // 2) * 128      # ft columns (pair blocks, rounded up)
    NSLICE = 4
    SL = WCOLS // NSLICE            # 432 cols -> one PSUM bank each
    nc = bacc.Bacc("TRN2", target_bir_lowering=False, debug=False,
                   enable_asserts=False, num_devices=N_CORES)
    ft = nc.dram_tensor("ft", [2 * CIN, FCOLS], mybir.dt.bfloat16,
                        kind="ExternalInput")
    wt = nc.dram_tensor("wt", [2 * CIN, WCOLS], mybir.dt.bfloat16,
                        kind="ExternalInput")
    work = nc.dram_tensor("work", [NPTS, WCOLS], mybir.dt.bfloat16,
                          kind="ExternalOutput")

    with tile.TileContext(nc) as tc:
        with (
            tc.tile_pool(name="const", bufs=1) as cpool,
            tc.tile_pool(name="cbuf", bufs=4) as cbpool,
            tc.tile_pool(name="psum", bufs=8 if style == "v6" else 4,
                         space="PSUM") as ppool,
        ):
            ft_t = cpool.tile([2 * CIN, FCOLS], mybir.dt.bfloat16)
            wt_t = cpool.tile([2 * CIN, WCOLS], mybir.dt.bfloat16)
            nc.sync.dma_start(out=wt_t[:], in_=wt[:])
            nseg = 6
            seg = -(-FCOLS // (128 * nseg)) * 128
            for s in range(nseg):
                s0, s1 = s * seg, min((s + 1) * seg, FCOLS)
                if s0 < s1:
                    nc.sync.dma_start(out=ft_t[:, s0:s1], in_=ft[:, s0:s1])

            for pr in range(NPAIR):
                cols = slice(pr * 128, (pr + 1) * 128)
                c_t = cbpool.tile([128, 2, WCOLS], mybir.dt.bfloat16)
                if style in ("v6", "v8"):
                    for j in range(4):
                        n = slice(j * SL, (j + 1) * SL)
                        for h in range(2):
                            hp = slice(h * CIN, (h + 1) * CIN)
                            ps = ppool.tile([128, SL], mybir.dt.float32,
                                            space="PSUM", tag="ps")
                            nc.tensor.matmul(
                                out=ps[:],
                                lhsT=ft_t[hp, cols],
                                rhs=wt_t[hp, n],
                                start=True, stop=True)
                            eng = (nc.vector.tensor_copy if (j + h) % 2 == 0
                                   else nc.scalar.copy)
                            eng(out=c_t[:, h, n], in_=ps[:])
                else:
                    for h in range(2):
                        hp = slice(h * CIN, (h + 1) * CIN)
                        for j in range(2):
                            w0 = j * 1024           # 1024 then 704 cols
                            w1 = min(w0 + 1024, WCOLS)
                            ps = ppool.tile([128, 1024], mybir.dt.float32,
                                            space="PSUM", tag="ps")
                            for m0 in range(w0, w1, 512):
                                m1 = min(m0 + 512, w1)
                                nc.tensor.matmul(
                                    out=ps[:, m0 - w0:m1 - w0],
                                    lhsT=ft_t[hp, cols],
                                    rhs=wt_t[hp, m0:m1],
                                    start=True, stop=True)
                            eng = (nc.vector.tensor_copy if (j + h) % 2 == 0
                                   else nc.scalar.copy)
                            eng(out=c_t[:, h, w0:w1], in_=ps[:, :w1 - w0])
                dst = work[pr * 256:(pr + 1) * 256, :].rearrange(
                    "(h q) c -> q h c", q=128)
                ends = (0, NPAIR - 1) if style == "v8" else (NPAIR,)
                if pr in ends:
                    # per-chunk DMAs at the pipeline ends: the first output
                    # flushes sooner and the drain waits on one chunk only
                    for h in range(2):
                        ch = 2 * pr + h
                        nc.sync.dma_start(
                            out=work[ch * 128:(ch + 1) * 128, :],
                            in_=c_t[:, h, :])
                elif pr < NPAIR - 1 or style == "v8":
                    nc.sync.dma_start(out=dst, in_=c_t[:])
                else:
                    # split the tail DMA so it starts before the last copies
                    nc.sync.dma_start(out=dst[:, :, :WCOLS // 2],
                                      in_=c_t[:, :, :WCOLS // 2])
                    nc.sync.dma_start(out=dst[:, :, WCOLS // 2:],
                                      in_=c_t[:, :, WCOLS // 2:])

            if style == "v8" and NCH % 2:
                # trailing singleton chunk rides the T0 half only
                ch = NCH - 1
                cols = slice(NPAIR * 128, (NPAIR + 1) * 128)
                c_t = cbpool.tile([128, 2, WCOLS], mybir.dt.bfloat16)
                for j in range(4):
                    n = slice(j * SL, (j + 1) * SL)
                    ps = ppool.tile([128, SL], mybir.dt.float32,
                                    space="PSUM", tag="ps")
                    nc.tensor.matmul(
                        out=ps[:],
                        lhsT=ft_t[:CIN, cols],
                        rhs=wt_t[:CIN, n],
                        start=True, stop=True)
                    eng = (nc.vector.tensor_copy if j % 2 == 0
                           else nc.scalar.copy)
                    eng(out=c_t[:, 0, n], in_=ps[:])
                nc.sync.dma_start(
                    out=work[ch * 128:(ch + 1) * 128, :], in_=c_t[:, 0, :])
    nc.compile()
    return nc


def kernel(feats, weight, bias, out_index, n_out):
    feats = np.asarray(feats, np.float32)
    weight = np.asarray(weight, np.float32)
    bias = np.asarray(bias, np.float32)
    oi = np.asarray(out_index, np.int32)
    n_out = int(n_out)

    # ---- merge duplicate-coordinate points (makes oi[k] unique per k) ----
    order = np.argsort(oi[0], kind="stable")
    b0 = oi[0][order]
    dup = np.zeros(len(order), bool)
    dup[1:] = b0[1:] == b0[:-1]
    heads = np.where(~dup, np.arange(len(order)), 0)
    np.maximum.accumulate(heads, out=heads)
    f_s = feats[order].copy()
    if dup.any():
        np.add.at(f_s, heads[dup], f_s[np.flatnonzero(dup)])
    keep = ~dup
    f_s = f_s[keep]
    oi_s = oi[:, order[keep]]                   # [27, M], unique per k
    M = oi_s.shape[1]

    # ---- shard points evenly across cores ----
    cnt = [(M + N_CORES - 1 - c) // N_CORES for c in range(N_CORES)]
    starts = np.cumsum([0] + cnt)
    NCH = -(-max(cnt) // 128)
    NCH += NCH % 2                  # chunk pairs
    FPTS = NCH * 128                # points padded to full chunk pairs

    if NCH not in _prog_cache:
        _prog_cache[NCH] = _build_program(NCH, "v6")
    nc = _prog_cache[NCH]

    # rhs[c, k*64+o] = weight[k, o, c]; duplicated on partitions 64-127
    wt_half = np.ascontiguousarray(
        weight.transpose(2, 0, 1).reshape(CIN, WCOLS)).astype(
            ml_dtypes.bfloat16)
    wt_np = np.concatenate([wt_half, wt_half])
    in_maps = []
    for c in range(N_CORES):
        fpad = np.zeros((FPTS, CIN), ml_dtypes.bfloat16)
        fpad[:cnt[c]] = f_s[starts[c]:starts[c + 1]].astype(ml_dtypes.bfloat16)
        # [npair, 2, 128, CIN] -> [2*CIN, npair * 128]
        ft_np = np.ascontiguousarray(
            fpad.reshape(FPTS // 256, 2, 128, CIN).transpose(1, 3, 0, 2)
            .reshape(2 * CIN, FPTS // 2))
        in_maps.append({"ft": ft_np, "wt": wt_np})

    res = run_bass_kernel_spmd(nc, in_maps, list(range(N_CORES)))
    _last["nc"] = nc
    _last["in_maps"] = in_maps

    # ---- host unshard: 27 exact per-offset merges + bias ----
    contrib = np.concatenate(
        [np.asarray(res.results[c]["work"])[:cnt[c]].reshape(cnt[c], KV, COUT)
         for c in range(N_CORES)])                 # [M, 27, 64] bf16
    out = np.empty((n_out, COUT), np.float32)
    out[:] = bias
    for k in range(KV):
        out[oi_s[k]] += contrib[:, k].astype(np.float32)
    return out


# revision 26
# speedup vs baseline: 1.2874x; 1.0015x over previous
"""Sparse ConvTranspose3d (gather + GEMM + scatter-add) on 8 TRN2 NeuronCores.

Device strategy: the per-token dma_scatter_add path is hard-capped by tiny
256B RMW packets to HBM (~280ns/packet: measured 4.7ms SWDGE-DMA busy in the
scatter baseline) and by Q7 descriptor generation (2.1ms), so the device
instead computes, for its shard of active voxels, all 27 per-offset GEMM
contributions and streams them to DRAM as large contiguous bf16 writes at
full DMA bandwidth. The index-directed scatter-add is part of the host-side
unshard: for each kernel offset k the (deduped) output indices are unique
(translation injectivity), so the merge is 27 exact vectorized
fancy-index adds.

Sharding: active voxels split evenly across 8 cores; weight replicated.

Per-core device pipeline (~86-95us, vs 7.16ms for the dma_scatter_add
baseline): chunk pairs of 128 points ride the PE's two 64-row tiles
(contraction=64), eight 1-bank PSUM tiles deep-pipeline the 8x432-col
matmuls per pair, PSUM evacuation alternates DVE/ACT, and each pair
flushes one 884KB contiguous bf16 DMA; DMA runs at ~350GB/s, the HBM
line rate, and is the critical path.
"""
import numpy as np
import ml_dtypes

import concourse.bass as bass
import concourse.bacc as bacc
import concourse.tile as tile
import concourse.mybir as mybir
from concourse.bass_utils import run_bass_kernel_spmd

N_CORES = 8
KV = 27
CIN = 64
COUT = 64
WCOLS = KV * COUT                # 1728

_prog_cache = {}
_last = {}                       # test-only: program + in_maps of last call


def _build_program(NCH, style="v7"):
    """SPMD program: [2*CIN, NPTS/2] paired feats -> [NPTS, KV*COUT].

    Chunk pairs share SBUF columns: even chunk's features live on
    partitions 0-63, odd chunk's on 64-127, so the two matmuls run
    concurrently on the PE's two 64-row tiles (T0 / T8).

    style "v6": eight 1-bank PSUM tiles/pair, eight 432-col copies.
    style "v7": four 2-bank PSUM tiles/pair, four 1024/704-col copies
    (512-col matmuls stay bank-aligned).
    style "v8": v6 PSUM layout; odd NCH allowed (trailing singleton chunk
    on the T0 half), and the first/last pairs flush per-chunk DMAs to
    tighten pipeline ramp and drain.
    """
    if style != "v8":
        assert NCH % 2 == 0
    NPTS = NCH * 128
    NPAIR = NCH // 2
    FCOLS = -(-NCH // 2) * 128      # ft columns (pair blocks, rounded up)
    NSLICE = 4
    SL = WCOLS // NSLICE            # 432 cols -> one PSUM bank each
    nc = bacc.Bacc("TRN2", target_bir_lowering=False, debug=False,
                   enable_asserts=False, num_devices=N_CORES)
    ft = nc.dram_tensor("ft", [2 * CIN, FCOLS], mybir.dt.bfloat16,
                        kind="ExternalInput")
    wt = nc.dram_tensor("wt", [2 * CIN, WCOLS], mybir.dt.bfloat16,
                        kind="ExternalInput")
    work = nc.dram_tensor("work", [NPTS, WCOLS], mybir.dt.bfloat16,
                          kind="ExternalOutput")

    with tile.TileContext(nc) as tc:
        with (
            tc.tile_pool(name="const", bufs=1) as cpool,
            tc.tile_pool(name="cbuf", bufs=4) as cbpool,
            tc.tile_pool(name="psum", bufs=8 if style == "v6" else 4,
                         space="PSUM") as ppool,
        ):
            ft_t = cpool.tile([2 * CIN, FCOLS], mybir.dt.bfloat16)
            wt_t = cpool.tile([2 * CIN, WCOLS], mybir.dt.bfloat16)
            nc.sync.dma_start(out=wt_t[:], in_=wt[:])
            nseg = 6
            seg = -(-FCOLS // (128 * nseg)) * 128
            bounds = [0, 128] + [min(128 + s * seg, FCOLS)
                                 for s in range(1, nseg + 1)]
            for s0, s1 in zip(bounds, bounds[1:]):
                if s0 < s1:
                    nc.sync.dma_start(out=ft_t[:, s0:s1], in_=ft[:, s0:s1])

            for pr in range(NPAIR):
                cols = slice(pr * 128, (pr + 1) * 128)
                c_t = cbpool.tile([128, 2, WCOLS], mybir.dt.bfloat16)
                if style in ("v6", "v8", "v9"):
                    for j in range(4):
                        n = slice(j * SL, (j + 1) * SL)
                        for h in range(2):
                            hp = slice(h * CIN, (h + 1) * CIN)
                            ps = ppool.tile([128, SL], mybir.dt.float32,
                                            space="PSUM", tag="ps")
                            nc.tensor.matmul(
                                out=ps[:],
                                lhsT=ft_t[hp, cols],
                                rhs=wt_t[hp, n],
                                start=True, stop=True)
                            eng = (nc.vector.tensor_copy if (j + h) % 2 == 0
                                   else nc.scalar.copy)
                            eng(out=c_t[:, h, n], in_=ps[:])
                else:
                    for h in range(2):
                        hp = slice(h * CIN, (h + 1) * CIN)
                        for j in range(2):
                            w0 = j * 1024           # 1024 then 704 cols
                            w1 = min(w0 + 1024, WCOLS)
                            ps = ppool.tile([128, 1024], mybir.dt.float32,
                                            space="PSUM", tag="ps")
                            for m0 in range(w0, w1, 512):
                                m1 = min(m0 + 512, w1)
                                nc.tensor.matmul(
                                    out=ps[:, m0 - w0:m1 - w0],
                                    lhsT=ft_t[hp, cols],
                                    rhs=wt_t[hp, m0:m1],
                                    start=True, stop=True)
                            eng = (nc.vector.tensor_copy if (j + h) % 2 == 0
                                   else nc.scalar.copy)
                            eng(out=c_t[:, h, w0:w1], in_=ps[:, :w1 - w0])
                dst = work[pr * 256:(pr + 1) * 256, :].rearrange(
                    "(h q) c -> q h c", q=128)
                if style == "v9" and pr in (0, NPAIR - 1):
                    # quarter DMAs at the pipeline ends: each 432-col slice
                    # flushes as soon as its two copies land (shorter
                    # fill at pr 0, shorter drain at the last pair)
                    for j in range(4):
                        n = slice(j * SL, (j + 1) * SL)
                        nc.sync.dma_start(out=dst[:, :, n], in_=c_t[:, :, n])
                    continue
                ends = (0, NPAIR - 1) if style == "v8" else (NPAIR,)
                if pr in ends:
                    # per-chunk DMAs at the pipeline ends: the first output
                    # flushes sooner and the drain waits on one chunk only
                    for h in range(2):
                        ch = 2 * pr + h
                        nc.sync.dma_start(
                            out=work[ch * 128:(ch + 1) * 128, :],
                            in_=c_t[:, h, :])
                elif pr < NPAIR - 1 or style == "v8":
                    nc.sync.dma_start(out=dst, in_=c_t[:])
                else:
                    # split the tail DMA so it starts before the last copies
                    nc.sync.dma_start(out=dst[:, :, :WCOLS // 2],
                                      in_=c_t[:, :, :WCOLS // 2])
                    nc.sync.dma_start(out=dst[:, :, WCOLS // 2:],
                                      in_=c_t[:, :, WCOLS // 2:])

            if style == "v8" and NCH % 2:
                # trailing singleton chunk rides the T0 half only
                ch = NCH - 1
                cols = slice(NPAIR * 128, (NPAIR + 1) * 128)
                c_t = cbpool.tile([128, 2, WCOLS], mybir.dt.bfloat16)
                for j in range(4):
                    n = slice(j * SL, (j + 1) * SL)
                    ps = ppool.tile([128, SL], mybir.dt.float32,
                                    space="PSUM", tag="ps")
                    nc.tensor.matmul(
                        out=ps[:],
                        lhsT=ft_t[:CIN, cols],
                        rhs=wt_t[:CIN, n],
                        start=True, stop=True)
                    eng = (nc.vector.tensor_copy if j % 2 == 0
                           else nc.scalar.copy)
                    eng(out=c_t[:, 0, n], in_=ps[:])
                nc.sync.dma_start(
                    out=work[ch * 128:(ch + 1) * 128, :], in_=c_t[:, 0, :])
    nc.compile()
    return nc


def kernel(feats, weight, bias, out_index, n_out):
    feats = np.asarray(feats, np.float32)
    weight = np.asarray(weight, np.float32)
    bias = np.asarray(bias, np.float32)
    oi = np.asarray(out_index, np.int32)
    n_out = int(n_out)

    # ---- merge duplicate-coordinate points (makes oi[k] unique per k) ----
    order = np.argsort(oi[0], kind="stable")
    b0 = oi[0][order]
    dup = np.zeros(len(order), bool)
    dup[1:] = b0[1:] == b0[:-1]
    heads = np.where(~dup, np.arange(len(order)), 0)
    np.maximum.accumulate(heads, out=heads)
    f_s = feats[order].copy()
    if dup.any():
        np.add.at(f_s, heads[dup], f_s[np.flatnonzero(dup)])
    keep = ~dup
    f_s = f_s[keep]
    oi_s = oi[:, order[keep]]                   # [27, M], unique per k
    M = oi_s.shape[1]

    # ---- shard points evenly across cores ----
    cnt = [(M + N_CORES - 1 - c) // N_CORES for c in range(N_CORES)]
    starts = np.cumsum([0] + cnt)
    NCH = -(-max(cnt) // 128)
    NCH += NCH % 2                  # chunk pairs
    FPTS = NCH * 128                # points padded to full chunk pairs

    if NCH not in _prog_cache:
        _prog_cache[NCH] = _build_program(NCH, "v6")
    nc = _prog_cache[NCH]

    # rhs[c, k*64+o] = weight[k, o, c]; duplicated on partitions 64-127
    wt_half = np.ascontiguousarray(
        weight.transpose(2, 0, 1).reshape(CIN, WCOLS)).astype(
            ml_dtypes.bfloat16)
    wt_np = np.concatenate([wt_half, wt_half])
    in_maps = []
    for c in range(N_CORES):
        fpad = np.zeros((FPTS, CIN), ml_dtypes.bfloat16)
        fpad[:cnt[c]] = f_s[starts[c]:starts[c + 1]].astype(ml_dtypes.bfloat16)
        # [npair, 2, 128, CIN] -> [2*CIN, npair * 128]
        ft_np = np.ascontiguousarray(
            fpad.reshape(FPTS // 256, 2, 128, CIN).transpose(1, 3, 0, 2)
            .reshape(2 * CIN, FPTS // 2))
        in_maps.append({"ft": ft_np, "wt": wt_np})

    res = run_bass_kernel_spmd(nc, in_maps, list(range(N_CORES)))
    _last["nc"] = nc
    _last["in_maps"] = in_maps

    # ---- host unshard: 27 exact per-offset merges + bias ----
    contrib = np.concatenate(
        [np.asarray(res.results[c]["work"])[:cnt[c]].reshape(cnt[c], KV, COUT)
         for c in range(N_CORES)])                 # [M, 27, 64] bf16
    out = np.empty((n_out, COUT), np.float32)
    out[:] = bias
    for k in range(KV):
        out[oi_s[k]] += contrib[:, k].astype(np.float32)
    return out
